# revision 1
# baseline (speedup 1.0000x reference)
"""Trainium2 Bass kernel for nn_Critic (MLP value function + GAE).

Sharding: batch B=2048 split across 8 NeuronCores (256 each). MLP params
replicated. The time recurrence (reverse GAE scan) is independent per batch
element, so no cross-core communication.

Strategy (single-pass bf16, ~500 us/core vs ~464 us PE streaming floor):
  - Host pre-transposes states to [D, T+1 * BC] bf16 per core, so the PE
    does zero transposes; DMA loads feature-major k-tiles directly.
  - Tokens (t, b) are flattened: 17*256 = 4352 tokens per core, processed
    in chunks of 512 (max fp32 PSUM bank / moving free size). All matmuls
    single-pass bf16 (1 cycle/row, FWL weight loads): end-to-end max relerr
    5.1e-3 vs the 2e-2 gate (matches the numpy bf16 simulation exactly).
  - W0 is host-reordered m-major so its first output-column blocks + the
    chunk-0 states land first and layer 0 starts within a few us of kernel
    start; W1/W2 stream in during chunk-0 compute (DMA priority ordering
    is empirically fragile: interleaving/splitting these loads regressed).
  - ELU(z) = min(exp(z)-1, relu(z)): ScalarE Exp + ScalarE Relu (both with
    fused +bias from PSUM), one VectorE combine writing bf16 directly.
  - value head: h3 (bf16) stationary [128 h, 128 tokens], Wo column moving
    -> psum [128 tokens, 1] accumulated over 8 k-tiles. Head matmuls for
    chunk c are deferred into chunk c+1's layer-0 stream so the PE never
    waits on the last ELU at a chunk boundary. ScalarE Identity with fused
    +bo writes valT [128 batch, 17 time] (stored time-reversed).
    (A fused head accumulating 4 token blocks as columns of one PSUM tile
    is WRONG: matmul start=True clears the whole PSUM bank, clobbering the
    other columns' accumulation groups.)
  - GAE: a handful of [128, 16/17] VectorE ops; the reverse scan is a
    single tensor_tensor_scan (state = dl*state + delta) since the host
    pre-reverses reward/cont and valT is written reversed.
"""

import sys

sys.path.insert(0, "/opt/trn_rl_repo")

import numpy as np

T, B, D, H = 16, 2048, 2048, 1024
NCORES = 8
BC = B // NCORES  # 256 batch per core
TP1 = T + 1
TOK = TP1 * BC  # 4352 tokens per core
DISCOUNT, LAMBDA = 0.99, 0.95
P = 128
KD = D // P  # 16 k-tiles for layer 0
KH = H // P  # 8 k-tiles for layers 1,2,out
MH = H // P  # 8 m-tiles of hidden units
CH = 512  # tokens per chunk (one PSUM bank of fp32)
NCH = (TOK + CH - 1) // CH  # 9 chunks: 8 full + 1 of 256

_NC_CACHE = None


def _build():
    import concourse.bacc as bacc
    import concourse.mybir as mybir
    from concourse.tile import TileContext

    F32 = mybir.dt.float32
    BF16 = mybir.dt.bfloat16
    ALU = mybir.AluOpType
    ACTF = mybir.ActivationFunctionType

    nc = bacc.Bacc(None, target_bir_lowering=False, debug=False)

    st_h = nc.declare_dram_parameter("statesT", [D, TOK], BF16, isOutput=False)
    rew_h = nc.declare_dram_parameter("rew_rev", [BC, T], F32, isOutput=False)
    cont_h = nc.declare_dram_parameter("cont_rev", [BC, TP1], F32, isOutput=False)
    # W0 host-reordered m-major: row m*P+p, col k*P+q  <-  W0[k*P+p, m*P+q],
    # so each output-column block m loads with ONE efficient DMA (4KB rows)
    w0_h = nc.declare_dram_parameter("W0", [MH * P, KD * P], BF16, isOutput=False)
    w1_h = nc.declare_dram_parameter("W1", [H, H], BF16, isOutput=False)
    w2_h = nc.declare_dram_parameter("W2", [H, H], BF16, isOutput=False)
    wo_h = nc.declare_dram_parameter("Wo", [P, KH], BF16, isOutput=False)
    b0_h = nc.declare_dram_parameter("b0", [P, MH], F32, isOutput=False)
    b1_h = nc.declare_dram_parameter("b1", [P, MH], F32, isOutput=False)
    b2_h = nc.declare_dram_parameter("b2", [P, MH], F32, isOutput=False)
    bo_h = nc.declare_dram_parameter("bo_b", [P, 1], F32, isOutput=False)
    ret_h = nc.declare_dram_parameter("ret_bt", [BC, T], F32, isOutput=True)
    val_h = nc.declare_dram_parameter("val_bt", [BC, T], F32, isOutput=True)

    with TileContext(nc) as tc:
        with (
            tc.tile_pool(name="wpool", bufs=1) as wpool,
            tc.tile_pool(name="xpool", bufs=3) as xpool,
            tc.tile_pool(name="hpool", bufs=2) as hpool,
            tc.tile_pool(name="tmp", bufs=4) as tmppool,
            tc.tile_pool(name="gae", bufs=1) as gaepool,
            tc.tile_pool(name="psA", bufs=5, space="PSUM") as psApool,
            tc.tile_pool(name="psV", bufs=2, space="PSUM") as psVpool,
        ):
            # ---- weights / constants ----
            # W0 is host-tiled [MH, KD, P, P] (m-major) so the 16 (k, m=0)
            # tiles + chunk-0 states (~2.5 MB) land first and layer 0 of the
            # first chunk starts within a few us; remaining W0 columns stream
            # in just ahead of the m-loop. W1/W2/biases load during chunk-0
            # compute; GAE inputs load during chunk 1.
            w0m = [
                wpool.tile([P, KD * P], BF16, name=f"w0m{m}", tag=f"w0m{m}")
                for m in range(MH)
            ]
            w1 = [
                wpool.tile([P, H], BF16, name=f"w1_{k}", tag=f"w1_{k}")
                for k in range(KH)
            ]
            w2 = [
                wpool.tile([P, H], BF16, name=f"w2_{k}", tag=f"w2_{k}")
                for k in range(KH)
            ]

            def load_xts(c, n):
                base = c * CH
                xts = []
                for k in range(KD):
                    xt = xpool.tile([P, CH], BF16, name=f"xt{k}", tag=f"xt{k}")
                    nc.sync.dma_start(
                        out=xt[:, :n],
                        in_=st_h[k * P : (k + 1) * P, base : base + n],
                    )
                    xts.append(xt)
                return xts

            def load_w0_col(m):
                nc.sync.dma_start(
                    out=w0m[m][:], in_=w0_h[m * P : (m + 1) * P, :]
                )

            # priority order: w0 cols 0-1 -> chunk-0 states (first half) ->
            # col 2 -> states (second half) -> col 3 -> biases -> cols 4..7
            load_w0_col(0)
            load_w0_col(1)
            xts0 = []
            for k in range(KD):
                xt = xpool.tile([P, CH], BF16, name=f"xt{k}", tag=f"xt{k}")
                nc.sync.dma_start(out=xt[:], in_=st_h[k * P : (k + 1) * P, 0:CH])
                xts0.append(xt)
                if k == 7:
                    load_w0_col(2)
            load_w0_col(3)
            bsb = []
            for li, bh in enumerate((b0_h, b1_h, b2_h)):
                bt = wpool.tile([P, MH], F32, name=f"bsb{li}", tag=f"bsb{li}")
                nc.sync.dma_start(out=bt[:], in_=bh[:])
                bsb.append(bt)
            for m in range(4, MH):
                load_w0_col(m)

            def load_rest():
                for tiles, dram_h in ((w1, w1_h), (w2, w2_h)):
                    for k in range(KH):
                        nc.sync.dma_start(
                            out=tiles[k][:], in_=dram_h[k * P : (k + 1) * P, :]
                        )
                wosb = wpool.tile([P, KH], BF16, name="wosb", tag="wosb")
                nc.sync.dma_start(out=wosb[:], in_=wo_h[:])
                bosb = wpool.tile([P, 1], F32, name="bosb", tag="bosb")
                nc.sync.dma_start(out=bosb[:], in_=bo_h[:])
                return wosb, bosb

            def load_gae_inputs():
                contsb = []
                rewsb = []
                for blk in range(2):
                    ct = gaepool.tile(
                        [P, TP1], F32, name=f"contsb{blk}", tag=f"contsb{blk}"
                    )
                    nc.sync.dma_start(
                        out=ct[:], in_=cont_h[blk * P : (blk + 1) * P, :]
                    )
                    contsb.append(ct)
                    rt = gaepool.tile([P, T], F32, name=f"rewsb{blk}", tag=f"rewsb{blk}")
                    nc.sync.dma_start(out=rt[:], in_=rew_h[blk * P : (blk + 1) * P, :])
                    rewsb.append(rt)
                return contsb, rewsb

            valT = []
            for blk in range(2):
                vt = gaepool.tile([P, TP1], F32, name=f"valT{blk}", tag=f"valT{blk}")
                valT.append(vt)

            wosb = bosb = contsb = rewsb = None

            # ---- chunked fused MLP over flattened (t, b) tokens ----
            pending_head = []
            for c in range(NCH):
                base = c * CH
                n = min(CH, TOK - base)
                xts = xts0 if c == 0 else load_xts(c, n)
                if c == 0:
                    wosb, bosb = load_rest()
                elif c == 1:
                    contsb, rewsb = load_gae_inputs()

                def layer(lhsT_of_km, nk, rhs_of_k, bias, houts, after_m=()):
                    for m in range(MH):
                        ps = psApool.tile([P, CH], F32, name="ps", tag="ps")
                        for k in range(nk):
                            nc.tensor.matmul(
                                ps[:, :n],
                                lhsT=lhsT_of_km(k, m),
                                rhs=rhs_of_k(k),
                                start=(k == 0),
                                stop=(k == nk - 1),
                                skip_group_check=True,
                            )
                        e = tmppool.tile([P, CH], F32, name="e", tag="e")
                        nc.scalar.activation(
                            e[:, :n], ps[:, :n], ACTF.Exp, bias=bias[:, m : m + 1]
                        )
                        rl = tmppool.tile([P, CH], F32, name="rl", tag="rl")
                        nc.scalar.activation(
                            rl[:, :n], ps[:, :n], ACTF.Relu, bias=bias[:, m : m + 1]
                        )
                        nc.vector.scalar_tensor_tensor(
                            houts[:, m * CH : m * CH + n],
                            e[:, :n],
                            1.0,
                            rl[:, :n],
                            ALU.subtract,
                            ALU.min,
                        )
                        if m < len(after_m):
                            after_m[m]()

                h1 = hpool.tile([P, MH * CH], BF16, name="h1", tag="h1")
                layer(
                    lambda k, m: w0m[m][:, k * P : (k + 1) * P],
                    KD,
                    lambda k: xts[k][:, :n],
                    bsb[0],
                    h1,
                    after_m=pending_head,
                )
                pending_head = []
                h2 = hpool.tile([P, MH * CH], BF16, name="h2", tag="h2")
                layer(
                    lambda k, m: w1[k][:, m * P : (m + 1) * P],
                    KH,
                    lambda k: h1[:, k * CH : k * CH + n],
                    bsb[1],
                    h2,
                )
                h3 = hpool.tile([P, MH * CH], BF16, name="h3", tag="h3")
                layer(
                    lambda k, m: w2[k][:, m * P : (m + 1) * P],
                    KH,
                    lambda k: h2[:, k * CH : k * CH + n],
                    bsb[2],
                    h3,
                )
                # value head: h3 stationary, Wo moving -> value [token, 1].
                # Deferred into the NEXT chunk's layer-0 stream so the PE can
                # jump straight from L2 into the next L0 without waiting for
                # the last ELU, and the head LDWs hide under N=512 streams.
                def make_head(c, h3, tb):
                    g = c * (CH // P) + tb  # global 128-token block
                    t_idx = g // 2
                    blk = g % 2

                    def emit():
                        pv = psVpool.tile([P, 1], F32, name="pv", tag="pv")
                        for k in range(KH):
                            nc.tensor.matmul(
                                pv[:],
                                lhsT=h3[:, k * CH + tb * P : k * CH + tb * P + P],
                                rhs=wosb[:, k : k + 1],
                                start=(k == 0),
                                stop=(k == KH - 1),
                                skip_group_check=True,
                            )
                        # store time-REVERSED: column 16-t, with fused +bo
                        nc.scalar.activation(
                            valT[blk][:, TP1 - 1 - t_idx : TP1 - t_idx],
                            pv[:],
                            ACTF.Identity,
                            bias=bosb[:],
                        )

                    return emit

                pending_head = [make_head(c, h3, tb) for tb in range(n // P)]

            # ---- GAE (all [128, 16/17] VectorE ops; time axis pre-reversed) ----
            # disc/dl depend only on cont; emit them before the final head
            # flush so only dtt/scan/ret trail the last value column (t=16,
            # stored reversed at column 0).
            discs, dls = [], []
            for blk in range(2):
                disc = gaepool.tile([P, T], F32, name=f"disc{blk}", tag=f"disc{blk}")
                nc.vector.tensor_scalar_mul(disc[:], contsb[blk][:, 0:T], DISCOUNT)
                dl = gaepool.tile([P, T], F32, name=f"dl{blk}", tag=f"dl{blk}")
                nc.vector.tensor_scalar_mul(dl[:], disc[:], LAMBDA)
                discs.append(disc)
                dls.append(dl)

            # flush the last chunk's head
            for emit in pending_head:
                emit()

            for blk in range(2):
                dtt = gaepool.tile([P, T], F32, name=f"dtt{blk}", tag=f"dtt{blk}")
                nc.vector.tensor_mul(dtt[:], discs[blk][:], valT[blk][:, 0:T])
                nc.vector.tensor_add(dtt[:], dtt[:], rewsb[blk][:])
                nc.vector.tensor_sub(dtt[:], dtt[:], valT[blk][:, 1:TP1])
                adv = gaepool.tile([P, T], F32, name=f"adv{blk}", tag=f"adv{blk}")
                nc.vector.tensor_tensor_scan(
                    adv[:], dls[blk][:], dtt[:], 0.0, ALU.mult, ALU.add
                )
                ret = gaepool.tile([P, T], F32, name=f"ret{blk}", tag=f"ret{blk}")
                nc.vector.tensor_add(ret[:], adv[:], valT[blk][:, 1:TP1])
                nc.sync.dma_start(out=ret_h[blk * P : (blk + 1) * P, :], in_=ret[:])
                nc.sync.dma_start(
                    out=val_h[blk * P : (blk + 1) * P, :], in_=valT[blk][:, 1:TP1]
                )

    nc.compile()
    return nc


def _get_nc():
    global _NC_CACHE
    if _NC_CACHE is None:
        _NC_CACHE = _build()
    return _NC_CACHE


def _make_in_maps(inputs):
    import ml_dtypes

    bf16 = ml_dtypes.bfloat16
    states = np.asarray(inputs["states"], dtype=np.float32)
    reward = np.asarray(inputs["reward"], dtype=np.float32)
    cont = np.asarray(inputs["cont"], dtype=np.float32)

    # [17, B, D] -> bf16 -> [D, 17, B] so per-core slices are token-major
    ST = np.ascontiguousarray(states.astype(bf16).transpose(2, 0, 1))

    # [D, H] -> [m, p, k, q]: one efficient DMA per output-column block m
    W0 = np.ascontiguousarray(
        np.asarray(inputs["W0"], np.float32)
        .astype(bf16)
        .reshape(KD, P, MH, P)
        .transpose(2, 1, 0, 3)
        .reshape(MH * P, KD * P)
    )
    W1 = np.ascontiguousarray(np.asarray(inputs["W1"], np.float32).astype(bf16))
    W2 = np.ascontiguousarray(np.asarray(inputs["W2"], np.float32).astype(bf16))
    Wo = np.ascontiguousarray(
        np.asarray(inputs["Wo"], np.float32).reshape(KH, P).T.astype(bf16)
    )
    b0 = np.ascontiguousarray(np.asarray(inputs["b0"], np.float32).reshape(MH, P).T)
    b1 = np.ascontiguousarray(np.asarray(inputs["b1"], np.float32).reshape(MH, P).T)
    b2 = np.ascontiguousarray(np.asarray(inputs["b2"], np.float32).reshape(MH, P).T)
    bo = np.ascontiguousarray(
        np.broadcast_to(np.asarray(inputs["bo"], np.float32).reshape(1, 1), (P, 1))
    )

    in_maps = []
    for c in range(NCORES):
        sl = slice(c * BC, (c + 1) * BC)
        in_maps.append(
            {
                "statesT": np.ascontiguousarray(ST[:, :, sl]).reshape(D, TOK),
                "rew_rev": np.ascontiguousarray(reward[::-1, sl].T),
                "cont_rev": np.ascontiguousarray(cont[::-1, sl].T),
                "W0": W0,
                "W1": W1,
                "W2": W2,
                "Wo": Wo,
                "b0": b0,
                "b1": b1,
                "b2": b2,
                "bo_b": bo,
            }
        )
    return in_maps


def _run(inputs, trace=False):
    try:
        import profhook

        profhook.ensure_hook()
    except Exception:
        pass
    from concourse.bass_utils import run_bass_kernel_spmd

    nc = _get_nc()
    in_maps = _make_in_maps(inputs)
    bkr = run_bass_kernel_spmd(nc, in_maps, list(range(NCORES)), trace=trace)
    ret = np.empty((T, B), np.float32)
    val = np.empty((T, B), np.float32)
    for c in range(NCORES):
        sl = slice(c * BC, (c + 1) * BC)
        ret[:, sl] = bkr.results[c]["ret_bt"].T[::-1]
        val[:, sl] = bkr.results[c]["val_bt"].T[::-1]
    return (ret, val), bkr


def kernel(**inputs):
    out, _ = _run(inputs, trace=False)
    return out

